# revision 29
# baseline (speedup 1.0000x reference)
"""Trainium2 Bass kernel for CrossCAM: cross channel-attention + 1x1 conv.

Reference computation (per batch b, C=64, N=H*W=16384):
    E_t = t_v @ t_v.T                     [C, C]   (t_v = template[b] as [C, N])
    E_r = r_v @ r_v.T
    attn_x = softmax(rowmax(E_x) - E_x)   rows; == exp(rowmin-E)/sum(exp(rowmin-E))
    t_out = gamma * (r_attn @ t_v) + t_v
    r_out = omega * (t_attn @ r_v) + r_v
    out   = conv_w @ concat(t_out, r_out) + conv_b        [64, N]

Key algebraic restructuring: the 1x1 conv distributes over the residual, so
    out = M_t @ t_v + M_r @ r_v + conv_b
    M_t = gamma * (w1 @ r_attn) + w1,   M_r = omega * (w2 @ t_attn) + w2
with w1 = conv_w[:, :64], w2 = conv_w[:, 64:].  Only ONE streaming pass over
the big tensors is needed; everything attention-related is 64x64.

Data layout on device ("split" layout): each [64, 16384] map is held in SBUF
as [128, 8192]: partition p = h*64+c holds t_v[c, h*8192:(h+1)*8192].  The
final matmul then runs with full K=128 using block-diagonal weights
W_x = blockdiag(M_xT, M_xT) [128, 128], and out128 in the same split layout
is contiguous-compatible with the HBM output tensor.

Sharding: pure data parallel, 2 batches per core on 8 cores.

When gamma == omega == 0 (the spec's input fill), M_t = w1 and M_r = w2 are
input constants: the attention pipeline is mathematically irrelevant (it is
multiplied by zero), so a fast program that skips it is exact.  The general
program computes the full attention path on device.
"""

import numpy as np

import concourse.bass as bass
import concourse.tile as tile
from concourse import bacc, mybir
from concourse import bass_utils

F32 = mybir.dt.float32
AX_X = mybir.AxisListType = mybir.AxisListType  # keep linters quiet

B, C, H, W = 16, 64, 128, 128
N = H * W          # 16384
NCORES = 8
BPC = B // NCORES  # batches per core
HALF = N // 2      # 8192
CK = 512           # matmul free-dim chunk
NCHUNK = HALF // CK  # 16

_programs: dict[tuple, object] = {}

# DMA engine knobs (A/B-tested on hardware):
#   "sync"/"scalar" = HWDGE rings, "gpsimd" = SWDGE
LOAD_ENGINE = "sync"
STORE_ENGINE = "scalar"
# PE dtype for the big streaming matmuls: "f32" (exact, 4 cyc/row) or
# "f32r" (relaxed fp32, 1 cyc/row at free-dim >= 256).
MM_DTYPE = "f32"
# Store chunk width in CK units (1 = per-bank stores, 2 = [128, 1024])
OC_WIDE = 2
# Fast path: quarters per map for pipelined loads
LQ = 4


def _qw():
    return HALF // LQ


def _build_program(with_attn: bool):
    nc = bacc.Bacc(
        "TRN2",
        target_bir_lowering=False,
        debug=False,
        enable_asserts=False,
        num_devices=NCORES,
    )
    # float32r = same 4-byte fp32 bits, but the PE runs 1 cycle/row (vs 4
    # for strict fp32) at free-dim >= 256, with relaxed internal rounding.
    # The whole produce-consume chain must carry the dtype.
    MMDT = (
        mybir.dt.float32r if (MM_DTYPE == "f32r" and not with_attn) else F32
    )
    t_in = nc.dram_tensor("t_in", [BPC, C, N], MMDT, kind="ExternalInput").ap()
    r_in = nc.dram_tensor("r_in", [BPC, C, N], MMDT, kind="ExternalInput").ap()
    wt0 = nc.dram_tensor("wt0", [128, 128], MMDT, kind="ExternalInput").ap()
    wr0 = nc.dram_tensor("wr0", [128, 128], MMDT, kind="ExternalInput").ap()
    bias2 = nc.dram_tensor("bias2", [128, 1], F32, kind="ExternalInput").ap()
    if with_attn:
        cwt1_d = nc.dram_tensor("cwt1", [C, C], F32, kind="ExternalInput").ap()
        cwt2_d = nc.dram_tensor("cwt2", [C, C], F32, kind="ExternalInput").ap()
        gam_d = nc.dram_tensor("gam2", [128, 1], F32, kind="ExternalInput").ap()
        omg_d = nc.dram_tensor("omg2", [128, 1], F32, kind="ExternalInput").ap()
        ident_d = nc.dram_tensor("ident", [128, 128], F32, kind="ExternalInput").ap()
    out = nc.dram_tensor("out", [BPC, C, N], F32, kind="ExternalOutput").ap()

    Exp = mybir.ActivationFunctionType.Exp
    Ident = mybir.ActivationFunctionType.Identity

    with tile.TileContext(nc) as tc:
        from contextlib import ExitStack

        with ExitStack() as ctx:
            const = ctx.enter_context(tc.tile_pool(name="const", bufs=1))
            vpool = ctx.enter_context(tc.tile_pool(name="v", bufs=2))
            pspool = ctx.enter_context(
                tc.tile_pool(name="ps", bufs=8 if not with_attn else 4, space="PSUM")
            )
            ocpool = ctx.enter_context(tc.tile_pool(name="oc", bufs=4))
            if with_attn:
                tppool = ctx.enter_context(tc.tile_pool(name="tp", bufs=2, space="PSUM"))
                egpool = ctx.enter_context(tc.tile_pool(name="eg", bufs=1, space="PSUM"))
                p1pool = ctx.enter_context(tc.tile_pool(name="p1", bufs=1, space="PSUM"))
                atpool = ctx.enter_context(tc.tile_pool(name="at", bufs=3))
                smpool = ctx.enter_context(tc.tile_pool(name="sm", bufs=2))

            Wt = const.tile([128, 128], MMDT, tag="Wt")
            nc.sync.dma_start(Wt[:], wt0[:])
            Wr = const.tile([128, 128], MMDT, tag="Wr")
            nc.sync.dma_start(Wr[:], wr0[:])
            bias_sb = const.tile([128, 1], F32, tag="bias")
            nc.sync.dma_start(bias_sb[:], bias2[:])
            if with_attn:
                cwt1 = const.tile([C, C], F32, tag="cwt1")
                nc.sync.dma_start(cwt1[:], cwt1_d[:])
                cwt2 = const.tile([C, C], F32, tag="cwt2")
                nc.sync.dma_start(cwt2[:], cwt2_d[:])
                gam = const.tile([128, 1], F32, tag="gam")
                nc.sync.dma_start(gam[:], gam_d[:])
                omg = const.tile([128, 1], F32, tag="omg")
                nc.sync.dma_start(omg[:], omg_d[:])
                ident = const.tile([128, 128], F32, tag="ident")
                nc.sync.dma_start(ident[:], ident_d[:])

            for i in range(BPC):
                # Load both maps in split layout [128, 8192]:
                # partition h*64+c <- v[c, h*8192 + n]
                # Two DMAs per map (one per half): 2D DRAM APs with outer
                # count 64 so HWDGE round-robins descriptors over all 16
                # SDMA engines (a 3D AP with outer count 2 lands on 2).
                ld = getattr(nc, LOAD_ENGINE if LOAD_ENGINE != "alt" else "sync")
                if with_attn:
                    # block-split layout: partition h*64+c <- v[c, h*HALF+n]
                    t128 = vpool.tile([128, HALF], MMDT, tag="t")
                    r128 = vpool.tile([128, HALF], MMDT, tag="r")
                    ld.dma_start(t128[0:64, :], t_in[i, :, 0:HALF])
                    ld.dma_start(t128[64:128, :], t_in[i, :, HALF:N])
                    ld.dma_start(r128[0:64, :], r_in[i, :, 0:HALF])
                    ld.dma_start(r128[64:128, :], r_in[i, :, HALF:N])
                else:
                    # interleaved layout: partition 2c+h <- v[c, h*HALF+n].
                    # One DMA covers all 128 partitions -> all 16 SBUF AXI
                    # ports engage concurrently (the split form above only
                    # drives half the ports per transfer).  Each map is
                    # loaded as LQ quarter tiles so the first matmuls can
                    # start as soon as the first quarter lands.
                    QW = _qw()
                    t_il = t_in[i].rearrange("c (h n) -> (c h) n", h=2)
                    r_il = r_in[i].rearrange("c (h n) -> (c h) n", h=2)
                    tq, rq = [], []
                    for q in range(LQ):
                        if LOAD_ENGINE == "alt":
                            ld = nc.sync if q % 2 == 0 else nc.scalar
                        tt = vpool.tile([128, QW], MMDT, tag=f"t{q}")
                        ld.dma_start(tt[:], t_il[:, QW * q : QW * (q + 1)])
                        tq.append(tt)
                        rr = vpool.tile([128, QW], MMDT, tag=f"r{q}")
                        ld.dma_start(rr[:], r_il[:, QW * q : QW * (q + 1)])
                        rq.append(rr)

                if with_attn:
                    attn = {}
                    for name, v128 in (("t", t128), ("r", r128)):
                        # E_grand[a, b] = sum_f v128[a, f] v128[b, f], via
                        # PE-transposed chunks; E = diag-fold of E_grand.
                        eg_ps = egpool.tile([128, 128], F32, tag="eg")
                        for g in range(HALF // CK):
                            tp = tppool.tile([128, CK], F32, tag="tp")
                            for q in range(4):
                                k = 4 * g + q
                                nc.tensor.transpose(
                                    tp[:, 128 * q : 128 * (q + 1)],
                                    v128[:, 128 * k : 128 * (k + 1)],
                                    ident[:],
                                )
                            at = atpool.tile([128, CK], F32, tag="at")
                            nc.scalar.copy(at[:], tp[:])
                            for q in range(4):
                                k = 4 * g + q
                                sl = at[:, 128 * q : 128 * (q + 1)]
                                nc.tensor.matmul(
                                    eg_ps[:],
                                    sl,
                                    sl,
                                    start=(k == 0),
                                    stop=(k == HALF // 128 - 1),
                                )
                        egs = smpool.tile([128, 128], F32, tag="egs")
                        nc.vector.tensor_copy(egs[:], eg_ps[:])
                        eglow = smpool.tile([C, C], F32, tag="eglow")
                        nc.sync.dma_start(eglow[:], egs[64:128, 64:128])
                        e = smpool.tile([C, C], F32, tag="e")
                        nc.vector.tensor_add(e[:], egs[0:64, 0:64], eglow[:])
                        # softmax(rowmax(E)-E) == exp(rowmin(E)-E)/sum(...)
                        rmin = smpool.tile([C, 1], F32, tag="rmin")
                        nc.vector.tensor_reduce(
                            rmin[:], e[:], axis=mybir.AxisListType.X,
                            op=mybir.AluOpType.min,
                        )
                        p = smpool.tile([C, C], F32, tag="p")
                        rsum = smpool.tile([C, 1], F32, tag="rsum")
                        nc.scalar.activation(
                            p[:], e[:], Exp, bias=rmin[:], scale=-1.0,
                            accum_out=rsum[:],
                        )
                        rinv = smpool.tile([C, 1], F32, tag="rinv")
                        nc.vector.reciprocal(rinv[:], rsum[:])
                        a = smpool.tile([C, C], F32, tag=f"attn_{name}")
                        nc.vector.tensor_scalar_mul(a[:], p[:], rinv[:])
                        attn[name] = a

                    # W_x diag blocks: M_tT = gamma*(w1@r_attn).T + w1T, etc.
                    # (w1@r_attn).T = r_attn.T.T @ w1T = matmul(lhsT=r_attn, rhs=w1T)
                    for wtile, a, cw, g_ap in (
                        (Wt, attn["r"], cwt1, gam),
                        (Wr, attn["t"], cwt2, omg),
                    ):
                        p1 = p1pool.tile([C, C], F32, tag="p1")
                        nc.tensor.matmul(p1[:], a[:], cw[:], start=True, stop=True)
                        tmp = smpool.tile([C, C], F32, tag="tmp")
                        nc.vector.tensor_scalar_mul(tmp[:], p1[:], g_ap[0:64, :])
                        nc.vector.tensor_add(wtile[0:64, 0:64], tmp[:], cw[:])
                        nc.sync.dma_start(wtile[64:128, 64:128], wtile[0:64, 0:64])

                # out128 = Wt.T @ t128 + Wr.T @ r128 + bias (same layout as v)
                st = getattr(nc, STORE_ENGINE)
                out_il = None
                if not with_attn:
                    out_il = out[i].rearrange("c (h n) -> (c h) n", h=2)

                def t_chunk(j):
                    if with_attn:
                        return t128[:, CK * j : CK * (j + 1)]
                    o = CK * j
                    qw = _qw()
                    return tq[o // qw][:, o % qw : o % qw + CK]

                def r_chunk(j):
                    if with_attn:
                        return r128[:, CK * j : CK * (j + 1)]
                    o = CK * j
                    qw = _qw()
                    return rq[o // qw][:, o % qw : o % qw + CK]

                group = max(_qw() // CK, OC_WIDE) if not with_attn else 4
                for g in range(NCHUNK // group):
                    pss = []
                    for q in range(group):
                        j = group * g + q
                        ps = pspool.tile([128, CK], F32, tag="ps")
                        nc.tensor.matmul(
                            ps[:], Wt[:], t_chunk(j),
                            start=True, stop=False,
                        )
                        pss.append((j, ps))
                    for j, ps in pss:
                        nc.tensor.matmul(
                            ps[:], Wr[:], r_chunk(j),
                            start=False, stop=True,
                        )
                    oc = None
                    for idx, (j, ps) in enumerate(pss):
                        w = idx % OC_WIDE
                        if w == 0:
                            oc = ocpool.tile([128, CK * OC_WIDE], F32, tag="oc")
                        nc.scalar.activation(
                            oc[:, CK * w : CK * (w + 1)], ps[:],
                            Ident, bias=bias_sb[:], scale=1.0,
                        )
                        if w < OC_WIDE - 1:
                            continue
                        j0 = j - (OC_WIDE - 1)
                        span = CK * OC_WIDE
                        if with_attn:
                            st.dma_start(
                                out[i, :, CK * j0 : CK * j0 + span],
                                oc[0:64, :],
                            )
                            st.dma_start(
                                out[i, :, HALF + CK * j0 : HALF + CK * j0 + span],
                                oc[64:128, :],
                            )
                        else:
                            st.dma_start(
                                out_il[:, CK * j0 : CK * j0 + span], oc[:]
                            )

    nc.compile()
    return nc


def _get_program(with_attn: bool):
    key = (with_attn, LOAD_ENGINE, STORE_ENGINE, MM_DTYPE, OC_WIDE)
    prog = _programs.get(key)
    if prog is None:
        prog = _build_program(with_attn)
        _programs[key] = prog
    return prog


def make_in_maps(template_map, roi_map, gamma, omega, conv_w, conv_b):
    """Host-side prep: per-core input dicts + which program variant to use."""
    template_map = np.ascontiguousarray(np.asarray(template_map, dtype=np.float32))
    roi_map = np.ascontiguousarray(np.asarray(roi_map, dtype=np.float32))
    conv_w = np.asarray(conv_w, dtype=np.float32)
    conv_b = np.asarray(conv_b, dtype=np.float32)
    g = float(np.asarray(gamma).reshape(-1)[0])
    o = float(np.asarray(omega).reshape(-1)[0])
    with_attn = not (g == 0.0 and o == 0.0)

    w1T = np.ascontiguousarray(conv_w[:, :C].T)  # [c, o]
    w2T = np.ascontiguousarray(conv_w[:, C:].T)
    if with_attn:
        # block-split layout: W[h*64+c, h*64+o] = wT[c, o]
        wt0 = np.zeros((128, 128), np.float32)
        wt0[:64, :64] = w1T
        wt0[64:, 64:] = w1T
        wr0 = np.zeros((128, 128), np.float32)
        wr0[:64, :64] = w2T
        wr0[64:, 64:] = w2T
        bias2 = np.ascontiguousarray(np.tile(conv_b, 2)[:, None])  # [128, 1]
    else:
        # interleaved layout: W[2c+h, 2o+h] = wT[c, o]
        eye2 = np.eye(2, dtype=np.float32)
        wt0 = np.ascontiguousarray(np.kron(w1T, eye2))
        wr0 = np.ascontiguousarray(np.kron(w2T, eye2))
        bias2 = np.ascontiguousarray(np.repeat(conv_b, 2)[:, None])

    common = {"wt0": wt0, "wr0": wr0, "bias2": bias2}
    if with_attn:
        common.update(
            cwt1=w1T,
            cwt2=w2T,
            gam2=np.full((128, 1), g, np.float32),
            omg2=np.full((128, 1), o, np.float32),
            ident=np.eye(128, dtype=np.float32),
        )

    tm = template_map.reshape(B, C, N)
    rm = roi_map.reshape(B, C, N)
    in_maps = [
        dict(
            common,
            t_in=tm[BPC * i : BPC * (i + 1)],
            r_in=rm[BPC * i : BPC * (i + 1)],
        )
        for i in range(NCORES)
    ]
    return in_maps, with_attn


def kernel(template_map, roi_map, gamma, omega, conv_w, conv_b):
    in_maps, with_attn = make_in_maps(
        template_map, roi_map, gamma, omega, conv_w, conv_b
    )
    nc = _get_program(with_attn)
    res = bass_utils.run_bass_kernel_spmd(nc, in_maps, core_ids=list(range(NCORES)))
    outp = np.concatenate([res.results[i]["out"] for i in range(NCORES)], axis=0)
    return outp.reshape(B, C, H, W)


# revision 30
# speedup vs baseline: 2.8336x; 2.8336x over previous
"""Trainium2 Bass kernel for CrossCAM: cross channel-attention + 1x1 conv.

Reference computation (per batch b, C=64, N=H*W=16384):
    E_t = t_v @ t_v.T                     [C, C]   (t_v = template[b] as [C, N])
    E_r = r_v @ r_v.T
    attn_x = softmax(rowmax(E_x) - E_x)   rows; == exp(rowmin-E)/sum(exp(rowmin-E))
    t_out = gamma * (r_attn @ t_v) + t_v
    r_out = omega * (t_attn @ r_v) + r_v
    out   = conv_w @ concat(t_out, r_out) + conv_b        [64, N]

Key algebraic restructuring: the 1x1 conv distributes over the residual, so
    out = M_t @ t_v + M_r @ r_v + conv_b
    M_t = gamma * (w1 @ r_attn) + w1,   M_r = omega * (w2 @ t_attn) + w2
with w1 = conv_w[:, :64], w2 = conv_w[:, 64:].  Only ONE streaming pass over
the big tensors is needed; everything attention-related is 64x64.

Data layout on device ("split" layout): each [64, 16384] map is held in SBUF
as [128, 8192]: partition p = h*64+c holds t_v[c, h*8192:(h+1)*8192].  The
final matmul then runs with full K=128 using block-diagonal weights
W_x = blockdiag(M_xT, M_xT) [128, 128], and out128 in the same split layout
is contiguous-compatible with the HBM output tensor.

Sharding: pure data parallel, 2 batches per core on 8 cores.

When gamma == omega == 0 (the spec's input fill), M_t = w1 and M_r = w2 are
input constants: the attention pipeline is mathematically irrelevant (it is
multiplied by zero), so a fast program that skips it is exact.  The general
program computes the full attention path on device.
"""

import numpy as np

import concourse.bass as bass
import concourse.tile as tile
from concourse import bacc, mybir
from concourse import bass_utils

F32 = mybir.dt.float32
AX_X = mybir.AxisListType = mybir.AxisListType  # keep linters quiet

B, C, H, W = 16, 64, 128, 128
N = H * W          # 16384
NCORES = 8
BPC = B // NCORES  # batches per core
HALF = N // 2      # 8192
CK = 512           # matmul free-dim chunk
NCHUNK = HALF // CK  # 16

_programs: dict[tuple, object] = {}

# DMA engine knobs (A/B-tested on hardware):
#   "sync"/"scalar" = HWDGE rings, "gpsimd" = SWDGE
LOAD_ENGINE = "sync"
STORE_ENGINE = "scalar"
# PE dtype for the big streaming matmuls: "f32" (exact, 4 cyc/row) or
# "f32r" (relaxed fp32, 1 cyc/row at free-dim >= 256).
MM_DTYPE = "f32"
# Store chunk width in CK units (1 = per-bank stores, 2 = [128, 1024])
OC_WIDE = 2
# Fast path: quarters per map for pipelined loads
LQ = 4


def _qw():
    return HALF // LQ


def _build_program(with_attn: bool):
    nc = bacc.Bacc(
        "TRN2",
        target_bir_lowering=False,
        debug=False,
        enable_asserts=False,
        num_devices=NCORES,
    )
    # float32r = same 4-byte fp32 bits, but the PE runs 1 cycle/row (vs 4
    # for strict fp32) at free-dim >= 256, with relaxed internal rounding.
    # The whole produce-consume chain must carry the dtype.
    MMDT = (
        mybir.dt.float32r if (MM_DTYPE == "f32r" and not with_attn) else F32
    )
    t_in = nc.dram_tensor("t_in", [BPC, C, N], MMDT, kind="ExternalInput").ap()
    r_in = nc.dram_tensor("r_in", [BPC, C, N], MMDT, kind="ExternalInput").ap()
    wt0 = nc.dram_tensor("wt0", [128, 128], MMDT, kind="ExternalInput").ap()
    wr0 = nc.dram_tensor("wr0", [128, 128], MMDT, kind="ExternalInput").ap()
    bias2 = nc.dram_tensor("bias2", [128, 1], F32, kind="ExternalInput").ap()
    if with_attn:
        cwt1_d = nc.dram_tensor("cwt1", [C, C], F32, kind="ExternalInput").ap()
        cwt2_d = nc.dram_tensor("cwt2", [C, C], F32, kind="ExternalInput").ap()
        gam_d = nc.dram_tensor("gam2", [128, 1], F32, kind="ExternalInput").ap()
        omg_d = nc.dram_tensor("omg2", [128, 1], F32, kind="ExternalInput").ap()
        ident_d = nc.dram_tensor("ident", [128, 128], F32, kind="ExternalInput").ap()
    out = nc.dram_tensor("out", [BPC, C, N], F32, kind="ExternalOutput").ap()

    Exp = mybir.ActivationFunctionType.Exp
    Ident = mybir.ActivationFunctionType.Identity

    with tile.TileContext(nc) as tc:
        from contextlib import ExitStack

        with ExitStack() as ctx:
            const = ctx.enter_context(tc.tile_pool(name="const", bufs=1))
            vpool = ctx.enter_context(tc.tile_pool(name="v", bufs=2))
            pspool = ctx.enter_context(
                tc.tile_pool(name="ps", bufs=8 if not with_attn else 4, space="PSUM")
            )
            ocpool = ctx.enter_context(tc.tile_pool(name="oc", bufs=4))
            if with_attn:
                tppool = ctx.enter_context(tc.tile_pool(name="tp", bufs=2, space="PSUM"))
                egpool = ctx.enter_context(tc.tile_pool(name="eg", bufs=1, space="PSUM"))
                p1pool = ctx.enter_context(tc.tile_pool(name="p1", bufs=1, space="PSUM"))
                atpool = ctx.enter_context(tc.tile_pool(name="at", bufs=3))
                smpool = ctx.enter_context(tc.tile_pool(name="sm", bufs=2))

            cld = nc.gpsimd if not with_attn else nc.sync
            Wt = const.tile([128, 128], MMDT, tag="Wt")
            cld.dma_start(Wt[:], wt0[:])
            Wr = const.tile([128, 128], MMDT, tag="Wr")
            cld.dma_start(Wr[:], wr0[:])
            bias_sb = const.tile([128, 1], F32, tag="bias")
            cld.dma_start(bias_sb[:], bias2[:])
            if with_attn:
                cwt1 = const.tile([C, C], F32, tag="cwt1")
                nc.sync.dma_start(cwt1[:], cwt1_d[:])
                cwt2 = const.tile([C, C], F32, tag="cwt2")
                nc.sync.dma_start(cwt2[:], cwt2_d[:])
                gam = const.tile([128, 1], F32, tag="gam")
                nc.sync.dma_start(gam[:], gam_d[:])
                omg = const.tile([128, 1], F32, tag="omg")
                nc.sync.dma_start(omg[:], omg_d[:])
                ident = const.tile([128, 128], F32, tag="ident")
                nc.sync.dma_start(ident[:], ident_d[:])

            for i in range(BPC):
                # Load both maps in split layout [128, 8192]:
                # partition h*64+c <- v[c, h*8192 + n]
                # Two DMAs per map (one per half): 2D DRAM APs with outer
                # count 64 so HWDGE round-robins descriptors over all 16
                # SDMA engines (a 3D AP with outer count 2 lands on 2).
                ld = getattr(nc, LOAD_ENGINE if LOAD_ENGINE != "alt" else "sync")
                if with_attn:
                    # block-split layout: partition h*64+c <- v[c, h*HALF+n]
                    t128 = vpool.tile([128, HALF], MMDT, tag="t")
                    r128 = vpool.tile([128, HALF], MMDT, tag="r")
                    ld.dma_start(t128[0:64, :], t_in[i, :, 0:HALF])
                    ld.dma_start(t128[64:128, :], t_in[i, :, HALF:N])
                    ld.dma_start(r128[0:64, :], r_in[i, :, 0:HALF])
                    ld.dma_start(r128[64:128, :], r_in[i, :, HALF:N])
                else:
                    # interleaved layout: partition 2c+h <- v[c, h*HALF+n].
                    # One DMA covers all 128 partitions -> all 16 SBUF AXI
                    # ports engage concurrently (the split form above only
                    # drives half the ports per transfer).  Each map is
                    # loaded as LQ quarter tiles so the first matmuls can
                    # start as soon as the first quarter lands.
                    QW = _qw()
                    t_il = t_in[i].rearrange("c (h n) -> (c h) n", h=2)
                    r_il = r_in[i].rearrange("c (h n) -> (c h) n", h=2)
                    tq, rq = [], []
                    for q in range(LQ):
                        if LOAD_ENGINE == "alt":
                            ld = nc.sync if q % 2 == 0 else nc.scalar
                        tt = vpool.tile([128, QW], MMDT, tag=f"t{q}")
                        ld.dma_start(tt[:], t_il[:, QW * q : QW * (q + 1)])
                        tq.append(tt)
                        rr = vpool.tile([128, QW], MMDT, tag=f"r{q}")
                        ld.dma_start(rr[:], r_il[:, QW * q : QW * (q + 1)])
                        rq.append(rr)

                if with_attn:
                    attn = {}
                    for name, v128 in (("t", t128), ("r", r128)):
                        # E_grand[a, b] = sum_f v128[a, f] v128[b, f], via
                        # PE-transposed chunks; E = diag-fold of E_grand.
                        eg_ps = egpool.tile([128, 128], F32, tag="eg")
                        for g in range(HALF // CK):
                            tp = tppool.tile([128, CK], F32, tag="tp")
                            for q in range(4):
                                k = 4 * g + q
                                nc.tensor.transpose(
                                    tp[:, 128 * q : 128 * (q + 1)],
                                    v128[:, 128 * k : 128 * (k + 1)],
                                    ident[:],
                                )
                            at = atpool.tile([128, CK], F32, tag="at")
                            nc.scalar.copy(at[:], tp[:])
                            for q in range(4):
                                k = 4 * g + q
                                sl = at[:, 128 * q : 128 * (q + 1)]
                                nc.tensor.matmul(
                                    eg_ps[:],
                                    sl,
                                    sl,
                                    start=(k == 0),
                                    stop=(k == HALF // 128 - 1),
                                )
                        egs = smpool.tile([128, 128], F32, tag="egs")
                        nc.vector.tensor_copy(egs[:], eg_ps[:])
                        eglow = smpool.tile([C, C], F32, tag="eglow")
                        nc.sync.dma_start(eglow[:], egs[64:128, 64:128])
                        e = smpool.tile([C, C], F32, tag="e")
                        nc.vector.tensor_add(e[:], egs[0:64, 0:64], eglow[:])
                        # softmax(rowmax(E)-E) == exp(rowmin(E)-E)/sum(...)
                        rmin = smpool.tile([C, 1], F32, tag="rmin")
                        nc.vector.tensor_reduce(
                            rmin[:], e[:], axis=mybir.AxisListType.X,
                            op=mybir.AluOpType.min,
                        )
                        p = smpool.tile([C, C], F32, tag="p")
                        rsum = smpool.tile([C, 1], F32, tag="rsum")
                        nc.scalar.activation(
                            p[:], e[:], Exp, bias=rmin[:], scale=-1.0,
                            accum_out=rsum[:],
                        )
                        rinv = smpool.tile([C, 1], F32, tag="rinv")
                        nc.vector.reciprocal(rinv[:], rsum[:])
                        a = smpool.tile([C, C], F32, tag=f"attn_{name}")
                        nc.vector.tensor_scalar_mul(a[:], p[:], rinv[:])
                        attn[name] = a

                    # W_x diag blocks: M_tT = gamma*(w1@r_attn).T + w1T, etc.
                    # (w1@r_attn).T = r_attn.T.T @ w1T = matmul(lhsT=r_attn, rhs=w1T)
                    for wtile, a, cw, g_ap in (
                        (Wt, attn["r"], cwt1, gam),
                        (Wr, attn["t"], cwt2, omg),
                    ):
                        p1 = p1pool.tile([C, C], F32, tag="p1")
                        nc.tensor.matmul(p1[:], a[:], cw[:], start=True, stop=True)
                        tmp = smpool.tile([C, C], F32, tag="tmp")
                        nc.vector.tensor_scalar_mul(tmp[:], p1[:], g_ap[0:64, :])
                        nc.vector.tensor_add(wtile[0:64, 0:64], tmp[:], cw[:])
                        nc.sync.dma_start(wtile[64:128, 64:128], wtile[0:64, 0:64])

                # out128 = Wt.T @ t128 + Wr.T @ r128 + bias (same layout as v)
                st = getattr(nc, STORE_ENGINE)
                out_il = None
                if not with_attn:
                    out_il = out[i].rearrange("c (h n) -> (c h) n", h=2)

                def t_chunk(j):
                    if with_attn:
                        return t128[:, CK * j : CK * (j + 1)]
                    o = CK * j
                    qw = _qw()
                    return tq[o // qw][:, o % qw : o % qw + CK]

                def r_chunk(j):
                    if with_attn:
                        return r128[:, CK * j : CK * (j + 1)]
                    o = CK * j
                    qw = _qw()
                    return rq[o // qw][:, o % qw : o % qw + CK]

                group = max(_qw() // CK, OC_WIDE) if not with_attn else 4
                for g in range(NCHUNK // group):
                    pss = []
                    for q in range(group):
                        j = group * g + q
                        ps = pspool.tile([128, CK], F32, tag="ps")
                        nc.tensor.matmul(
                            ps[:], Wt[:], t_chunk(j),
                            start=True, stop=False,
                        )
                        pss.append((j, ps))
                    for j, ps in pss:
                        nc.tensor.matmul(
                            ps[:], Wr[:], r_chunk(j),
                            start=False, stop=True,
                        )
                    oc = None
                    for idx, (j, ps) in enumerate(pss):
                        w = idx % OC_WIDE
                        if w == 0:
                            oc = ocpool.tile([128, CK * OC_WIDE], F32, tag="oc")
                        nc.scalar.activation(
                            oc[:, CK * w : CK * (w + 1)], ps[:],
                            Ident, bias=bias_sb[:], scale=1.0,
                        )
                        if w < OC_WIDE - 1:
                            continue
                        j0 = j - (OC_WIDE - 1)
                        span = CK * OC_WIDE
                        if with_attn:
                            st.dma_start(
                                out[i, :, CK * j0 : CK * j0 + span],
                                oc[0:64, :],
                            )
                            st.dma_start(
                                out[i, :, HALF + CK * j0 : HALF + CK * j0 + span],
                                oc[64:128, :],
                            )
                        else:
                            st.dma_start(
                                out_il[:, CK * j0 : CK * j0 + span], oc[:]
                            )

    nc.compile()
    return nc


def _get_program(with_attn: bool):
    key = (with_attn, LOAD_ENGINE, STORE_ENGINE, MM_DTYPE, OC_WIDE)
    prog = _programs.get(key)
    if prog is None:
        prog = _build_program(with_attn)
        _programs[key] = prog
    return prog


def make_in_maps(template_map, roi_map, gamma, omega, conv_w, conv_b):
    """Host-side prep: per-core input dicts + which program variant to use."""
    template_map = np.ascontiguousarray(np.asarray(template_map, dtype=np.float32))
    roi_map = np.ascontiguousarray(np.asarray(roi_map, dtype=np.float32))
    conv_w = np.asarray(conv_w, dtype=np.float32)
    conv_b = np.asarray(conv_b, dtype=np.float32)
    g = float(np.asarray(gamma).reshape(-1)[0])
    o = float(np.asarray(omega).reshape(-1)[0])
    with_attn = not (g == 0.0 and o == 0.0)

    w1T = np.ascontiguousarray(conv_w[:, :C].T)  # [c, o]
    w2T = np.ascontiguousarray(conv_w[:, C:].T)
    if with_attn:
        # block-split layout: W[h*64+c, h*64+o] = wT[c, o]
        wt0 = np.zeros((128, 128), np.float32)
        wt0[:64, :64] = w1T
        wt0[64:, 64:] = w1T
        wr0 = np.zeros((128, 128), np.float32)
        wr0[:64, :64] = w2T
        wr0[64:, 64:] = w2T
        bias2 = np.ascontiguousarray(np.tile(conv_b, 2)[:, None])  # [128, 1]
    else:
        # interleaved layout: W[2c+h, 2o+h] = wT[c, o]
        eye2 = np.eye(2, dtype=np.float32)
        wt0 = np.ascontiguousarray(np.kron(w1T, eye2))
        wr0 = np.ascontiguousarray(np.kron(w2T, eye2))
        bias2 = np.ascontiguousarray(np.repeat(conv_b, 2)[:, None])

    common = {"wt0": wt0, "wr0": wr0, "bias2": bias2}
    if with_attn:
        common.update(
            cwt1=w1T,
            cwt2=w2T,
            gam2=np.full((128, 1), g, np.float32),
            omg2=np.full((128, 1), o, np.float32),
            ident=np.eye(128, dtype=np.float32),
        )

    tm = template_map.reshape(B, C, N)
    rm = roi_map.reshape(B, C, N)
    in_maps = [
        dict(
            common,
            t_in=tm[BPC * i : BPC * (i + 1)],
            r_in=rm[BPC * i : BPC * (i + 1)],
        )
        for i in range(NCORES)
    ]
    return in_maps, with_attn


def kernel(template_map, roi_map, gamma, omega, conv_w, conv_b):
    in_maps, with_attn = make_in_maps(
        template_map, roi_map, gamma, omega, conv_w, conv_b
    )
    nc = _get_program(with_attn)
    res = bass_utils.run_bass_kernel_spmd(nc, in_maps, core_ids=list(range(NCORES)))
    outp = np.concatenate([res.results[i]["out"] for i in range(NCORES)], axis=0)
    return outp.reshape(B, C, H, W)
